# revision 1
# baseline (speedup 1.0000x reference)
"""Trainium2 Bass kernel for nn_CICDM — pair-feature reformulation.

Math: the Choquet integral C[n,b] is linear in shared features
  F = [R (435 pair hinges), sel (30), U (1024 per-exercise triple mins)]
  R[p=(i<j)] = relu(sel_i - sel_j)
  U[n] = min(R[p02(n)], R[p12(n)]) = relu(min(d02, d12))
so layer-1 of the MLP folds the whole per-exercise coefficient structure
into a host-precomputed W1F = w1 @ Gamma^T:  z1 = W1F @ F + b1.
The device never materializes C.

Per core (batch 512, split in two 256-col halves for tail overlap):
  - indirect-gather emb rows, sigmoid, PE-transpose -> selS strips (x4 row
    groups for 4-way concurrent K=30 strip matmuls)
  - pairs: 4 strip matmuls -> PSUM -> relu -> R tiles (l1 rhs chunks 0-3)
  - per exercise tile t: E02/E12 strip matmuls -> r02 = relu(E02) (pool),
    U_t = (E12 max 0) min r02 (one DVE STT) -> l1 rhs chunks 4-11
  - l1 accumulates 12 k-chunks x 2 m-tiles in PSUM; relu+bias -> h1
  - l2, relu -> h2; l3 per 128-exercise tile + ACT sigmoid -> fp16 out DMA
"""

import numpy as np

B = 4096
NCORES = 8
BL = B // NCORES          # 512 local batch
NH = 2                    # batch halves
HC = BL // NH             # 256 cols per half
KN = 30
NOUT = 1024
NT = NOUT // 128          # 8 exercise tiles
P = 128
NG = BL // P              # 4 gather groups (128 rows each)
S_N = 100000
N_WARM = 10

_PROG_CACHE = {}


def _np_f16():
    import ml_dtypes
    return np.dtype(ml_dtypes.bfloat16)


def _host_prep(q_idx, fm_vars, w1, b1, w2, b2, w3, b3):
    """Pair tables + folded W1F + packed weight layouts (all host-side)."""
    f16 = _np_f16()
    q = np.asarray(q_idx).astype(np.int64)            # [1024, 3] sorted asc
    fm = np.asarray(fm_vars, dtype=np.float64)
    w1 = np.asarray(w1, np.float64)

    chi = np.abs(fm)
    f0, f1, f3 = chi[0], chi[1], chi[3]
    F0 = np.minimum(f0, 1.0)
    F1 = np.minimum(f1, 1.0)
    F2 = np.minimum(np.maximum(f0, f1) + chi[2], 1.0)
    F3 = np.minimum(f3, 1.0)
    F4 = np.minimum(np.maximum(f3, f0) + chi[4], 1.0)
    F5 = np.minimum(np.maximum(f3, f1) + chi[5], 1.0)
    m0, m1, m3 = F0, F1, F3
    m2 = F2 - F0 - F1
    m4 = F4 - F0 - F3
    m5 = F5 - F1 - F3
    m6 = 1.0 - F2 - F4 - F5 + F0 + F1 + F3
    # C = c0 x0 + c1 x1 + c2 x2 + a01 r01 + a02 r02 + a12 r12 + aU min(r02,r12)
    c0 = m0 + m2 + m4 + m6
    c1 = m1 + m5
    c2 = m3
    a01 = -(m2 + m6)
    a02 = -m4
    a12 = -m5
    aU = -m6

    # pair table (ordered pairs i<j as they appear; q columns sorted asc)
    pairs = {}

    def pid(i, j):
        key = (int(i), int(j))
        if key not in pairs:
            pairs[key] = len(pairs)
        return pairs[key]

    p01 = np.array([pid(q[n, 0], q[n, 1]) for n in range(NOUT)])
    p02 = np.array([pid(q[n, 0], q[n, 2]) for n in range(NOUT)])
    p12 = np.array([pid(q[n, 1], q[n, 2]) for n in range(NOUT)])
    NP = len(pairs)                                    # ~435
    PI = np.empty(NP, np.int64)
    PJ = np.empty(NP, np.int64)
    for (i, j), p in pairs.items():
        PI[p], PJ[p] = i, j

    # gp: pair strips. tile s holds pairs [128s .. 128s+cols), strip rows at
    # partitions 32s (4-way row-group concurrency). [128, 4*128] fp16.
    n_ptile = (NP + P - 1) // P                        # 4
    assert n_ptile == 4 and NP - 3 * P <= 51 + 20
    gp = np.zeros((P, 4 * P), np.float32)
    for p in range(NP):
        s, c = p // P, p % P
        gp[32 * s + PI[p], s * P + c] += 1.0
        gp[32 * s + PJ[p], s * P + c] -= 1.0
    gp = gp.astype(f16)

    # gu: per-exercise-tile E-strips. slot idx = 2t+pl (pl 0->d02, 1->d12);
    # partition strip idx%4, col block (idx//4)*128. [128, 4*128] fp16.
    gu = np.zeros((P, 4 * P), np.float32)
    for t in range(NT):
        for pl in range(2):
            idx = 2 * t + pl
            sp, cb = idx % 4, idx // 4
            nn = np.arange(t * P, (t + 1) * P)
            src = q[nn, 0] if pl == 0 else q[nn, 1]
            gu[32 * sp + src, cb * P + (nn % P)] += 1.0
            gu[32 * sp + q[nn, 2], cb * P + (nn % P)] -= 1.0
    gu = gu.astype(f16)

    # W1F fold: features order = [R(0..NP-1); sel(30); U(1024)]
    KF_R = NP                                          # 435
    W1F = np.zeros((256, KF_R + KN + NOUT), np.float64)
    np.add.at(W1F.T, p01, (a01 * w1).T)
    np.add.at(W1F.T, p02, (a02 * w1).T)
    np.add.at(W1F.T, p12, (a12 * w1).T)
    for k, c in enumerate((c0, c1, c2)):
        np.add.at(W1F.T, KF_R + q[:, k], (c * w1).T)
    W1F[:, KF_R + KN:] = aU * w1

    # chunk packing [128, 12*256]: chunks 0-2 = R rows 0..383;
    # chunk 3 (K=81) = [R 384..434 (51); sel (30)]; chunks 4-11 = U tiles.
    w1f = np.zeros((P, 12 * 256), np.float32)
    for j in range(3):
        w1f[:, j * 256:(j + 1) * 256] = W1F[:, j * P:(j + 1) * P].T
    w1f[0:51, 3 * 256:4 * 256] = W1F[:, 3 * P:NP].T
    w1f[64:94, 3 * 256:4 * 256] = W1F[:, NP:NP + KN].T
    for t in range(NT):
        w1f[:, (4 + t) * 256:(5 + t) * 256] = \
            W1F[:, NP + KN + t * P:NP + KN + (t + 1) * P].T
    w1f = w1f.astype(f16)

    w2t = np.asarray(w2, np.float32).T.reshape(2, P, P)     # [k, p, o]
    w2s = np.ascontiguousarray(
        w2t.transpose(1, 0, 2).reshape(P, 2 * P)).astype(f16)
    w3s = np.ascontiguousarray(np.asarray(w3, np.float32).T).astype(f16)
    b1c = np.ascontiguousarray(np.asarray(b1, np.float32).reshape(2, P).T)
    b2c = np.ascontiguousarray(np.asarray(b2, np.float32).reshape(1, P).T)
    b3c = np.ascontiguousarray(np.asarray(b3, np.float32).reshape(NT, P).T)

    return dict(gp=gp, gu=gu, w1f=w1f, w2s=w2s, w3s=w3s,
                b1c=b1c, b2c=b2c, b3c=b3c)


def _build_program():
    key = "v2"
    if key in _PROG_CACHE:
        return _PROG_CACHE[key]

    import concourse.bacc as bacc
    import concourse.bass as bass
    import concourse.mybir as mybir
    import concourse.tile as tile
    from concourse.masks import make_identity

    f32 = mybir.dt.float32
    f16 = mybir.dt.bfloat16
    AF = mybir.ActivationFunctionType
    ALU = mybir.AluOpType

    nc = bacc.Bacc("TRN2", target_bir_lowering=False, debug=False,
                   num_swdge_queues=4)

    emb_d = nc.dram_tensor("emb", [S_N, KN], f32, kind="ExternalInput").ap()
    sidx_d = nc.dram_tensor("sidx", [P, NG], mybir.dt.int32,
                            kind="ExternalInput").ap()
    gp_d = nc.dram_tensor("gp", [P, 4 * P], f16, kind="ExternalInput").ap()
    gu_d = nc.dram_tensor("gu", [P, 4 * P], f16, kind="ExternalInput").ap()
    w1f_d = nc.dram_tensor("w1f", [P, 12 * 256], f16,
                           kind="ExternalInput").ap()
    w2_d = nc.dram_tensor("w2s", [P, 2 * P], f16, kind="ExternalInput").ap()
    w3_d = nc.dram_tensor("w3s", [P, NOUT], f16, kind="ExternalInput").ap()
    b1_d = nc.dram_tensor("b1c", [P, 2], f32, kind="ExternalInput").ap()
    b2_d = nc.dram_tensor("b2c", [P, 1], f32, kind="ExternalInput").ap()
    b3_d = nc.dram_tensor("b3c", [P, NT], f32, kind="ExternalInput").ap()
    out_d = nc.dram_tensor("out", [P, NT * (BL // 2)], f32,
                           kind="ExternalOutput").ap()

    def mm(out, lhsT, rhs, start, stop, tile_position=None):
        nc.tensor.matmul(out, lhsT, rhs, start=start, stop=stop,
                         tile_position=tile_position)

    with tile.TileContext(nc) as tc:
        with (
            tc.tile_pool(name="const", bufs=1) as cpool,
            tc.tile_pool(name="work", bufs=4) as wpool,
            tc.tile_pool(name="ptr", bufs=1, space="PSUM") as ptr,
            tc.tile_pool(name="pgen", bufs=4, space="PSUM") as pgen,
            tc.tile_pool(name="pl1", bufs=2, space="PSUM") as pl1,
            tc.tile_pool(name="pml", bufs=1, space="PSUM") as pml,
        ):
            # ---- index DMA + gathers first (longest latency chain) ----
            sidx_s = cpool.tile([P, NG], mybir.dt.int32, tag="sidx")
            nc.gpsimd.dma_start(sidx_s[:], sidx_d[:])
            stu4 = cpool.tile([P, NG * KN], f32, tag="stu4")
            for g in range(NG):
                nc.gpsimd.indirect_dma_start(
                    out=stu4[:, g * KN:(g + 1) * KN], out_offset=None,
                    in_=emb_d[:],
                    in_offset=bass.IndirectOffsetOnAxis(
                        ap=sidx_s[:, g:g + 1], axis=0))

            # ---- PE warm-up while DMAs land ----
            warm = cpool.tile([32, BL], f16, tag="warm")
            nc.vector.memset(warm[:], 0.0)
            wps = pml.tile([P, BL], f32, tag="ml")
            for _ in range(N_WARM):
                mm(wps[0:32, 0:HC], warm[0:32, 0:32], warm[0:32, 0:HC],
                   True, True, tile_position=(0, 0))

            # ---- weights in ----
            gp_s = cpool.tile([P, 4 * P], f16, tag="gp")
            nc.sync.dma_start(gp_s[:], gp_d[:])
            gu_s = cpool.tile([P, 4 * P], f16, tag="gu")
            nc.sync.dma_start(gu_s[:], gu_d[:])
            w1f_s = cpool.tile([P, 12 * 256], f16, tag="w1f")
            nc.gpsimd.dma_start(w1f_s[:], w1f_d[:])
            w2_s = cpool.tile([P, 2 * P], f16, tag="w2")
            nc.sync.dma_start(w2_s[:], w2_d[:])
            w3_s = cpool.tile([P, NOUT], f16, tag="w3")
            nc.gpsimd.dma_start(w3_s[:], w3_d[:])
            b1_s = cpool.tile([P, 2], f32, tag="b1")
            nc.sync.dma_start(b1_s[:], b1_d[:])
            b2_s = cpool.tile([P, 1], f32, tag="b2")
            nc.sync.dma_start(b2_s[:], b2_d[:])
            b3_s = cpool.tile([P, NT], f32, tag="b3")
            nc.sync.dma_start(b3_s[:], b3_d[:])

            ident = cpool.tile([P, P], f16, tag="ident")
            make_identity(nc, ident[:])
            # preload ACT table early (overlaps DMA wait)
            dum = cpool.tile([P, 2], f32, tag="dum")
            nc.vector.memset(dum[:, 0:1], 0.0)
            nc.scalar.activation(dum[:, 1:2], dum[:, 0:1], AF.Sigmoid)
            osb_big = cpool.tile([P, NT * BL], f16, tag="osb_big")

            # ---- per-group sigmoid -> transpose -> strip copies ----
            # selS split into 4 tiles (strip s at partitions 32s) so the
            # copies don't serialize on tile-granular WAW tracking.
            sel4 = cpool.tile([P, NG * KN], f16, tag="sel4")
            for g in range(NG):
                nc.scalar.activation(sel4[:, g * KN:(g + 1) * KN],
                                     stu4[:, g * KN:(g + 1) * KN], AF.Sigmoid)
            tp = ptr.tile([32, BL], f16, tag="tp")
            for g in range(NG):
                nc.tensor.transpose(tp[0:KN, g * P:(g + 1) * P],
                                    sel4[:, g * KN:(g + 1) * KN], ident[:])
            selSt = [cpool.tile([P, BL], f16, tag=f"selS{s}", name=f"selS{s}")
                     for s in range(4)]
            mix = cpool.tile([P, BL], f16, tag="mix")
            copy_eng = [nc.vector, nc.scalar, nc.vector, nc.scalar]
            for g in range(NG):
                for s in range(4):
                    copy_eng[s].__class__  # noqa
                for s in range(4):
                    e = copy_eng[s]
                    if e is nc.vector:
                        e.tensor_copy(
                            selSt[s][32 * s:32 * s + KN, g * P:(g + 1) * P],
                            tp[0:KN, g * P:(g + 1) * P])
                    else:
                        e.copy(
                            selSt[s][32 * s:32 * s + KN, g * P:(g + 1) * P],
                            tp[0:KN, g * P:(g + 1) * P])
            nc.vector.memset(mix[32:64, :], 0.0)
            nc.vector.tensor_copy(mix[64:64 + KN, :], tp[0:KN, :])

            def selS(sl, cols):
                return selSt[sl.start // 32][sl, cols]

            # ---- per-half pipeline ----
            R_tiles = [cpool.tile([P, BL], f16, tag=f"R{s}", name=f"R{s}")
                       for s in range(3)]
            U_tiles = [cpool.tile([P, BL], f16, tag=f"U{t}", name=f"U{t}")
                       for t in range(NT)]
            h1 = cpool.tile([P, 2 * BL], f16, tag="h1")
            h2 = cpool.tile([P, BL], f16, tag="h2")
            l1ps = {}
            PCOLS = [51, 51, 51, 51]  # R remainder rows in Dp tile 3

            def front_full():
                # pairs: 4 concurrent strips, one full bank each
                dps = []
                for s in range(4):
                    cols = P if s < 3 else 51
                    dp = pgen.tile([P, BL], f32, tag="g", name=f"dp{s}")
                    mm(dp[0:cols, :], gp_s[32 * s:32 * s + KN,
                                           s * P:s * P + cols],
                       selSt[s][32 * s:32 * s + KN, :], True, True,
                       tile_position=(32 * s, 0))
                    dps.append(dp)
                # R relus (PSUM -> SBUF fp16: DVE/ACT only)
                nc.vector.tensor_scalar(R_tiles[0][:, :], dps[0][:],
                                        0.0, None, ALU.max)
                nc.vector.tensor_scalar(R_tiles[1][:, :], dps[1][:],
                                        0.0, None, ALU.max)
                nc.scalar.activation(R_tiles[2][:, :], dps[2][:], AF.Relu)
                nc.scalar.activation(mix[0:51, :], dps[3][0:51, :], AF.Relu)

            def l1_chunk(j, rhs_ap, kj):
                for m in range(2):
                    if m not in l1ps:
                        l1ps[m] = pl1.tile([P, BL], f32, tag="l1",
                                           name=f"l1_{m}")
                    mm(l1ps[m][:, :],
                       w1f_s[0:kj, j * 256 + m * P:j * 256 + m * P + P],
                       rhs_ap, j == 0, j == 11)

            def u_tile(t):
                eb = []
                for pl in range(2):
                    idx = 2 * t + pl
                    sp, cb = idx % 4, idx // 4
                    ep = pgen.tile([P, BL], f32, tag="g", name=f"e{t}{pl}")
                    mm(ep[:], gu_s[32 * sp:32 * sp + KN, cb * P:(cb + 1) * P],
                       selSt[sp][32 * sp:32 * sp + KN, :], True, True,
                       tile_position=(32 * sp, 0))
                    eb.append(ep)
                # r02 = relu(E02); U = (E12 max 0) min r02
                r02 = wpool.tile([P, BL], f16, tag="r02")
                nc.scalar.activation(r02[:], eb[0][:], AF.Relu)
                nc.vector.scalar_tensor_tensor(
                    U_tiles[t][:, :], eb[1][:], 0.0, r02[:],
                    ALU.max, ALU.min)

            def mlp_head():
                for m in range(2):
                    nc.scalar.activation(h1[:, m * BL:(m + 1) * BL],
                                         l1ps[m][:, :],
                                         AF.Relu, bias=b1_s[:, m:m + 1])

            def mlp_l2():
                l2p = pml.tile([P, BL], f32, tag="ml", name="l2")
                mm(l2p[:], w2_s[:, 0:P], h1[:, 0:BL], True, False)
                mm(l2p[:], w2_s[:, P:2 * P], h1[:, BL:2 * BL], False, True)
                nc.scalar.activation(h2[:], l2p[:], AF.Relu,
                                     bias=b2_s[:, 0:1])

            def mlp_l3(o):
                bank = pgen.tile([P, BL], f32, tag="g", name=f"l3_{o}")
                mm(bank[:], w3_s[:, o * P:(o + 1) * P], h2[:], True, True)
                nc.scalar.activation(
                    osb_big[:, o * BL:(o + 1) * BL],
                    bank[:], AF.Sigmoid, bias=b3_s[:, o:o + 1])

            def out_dma(o0, o1):
                nc.sync.dma_start(
                    out_d[:, o0 * (BL // 2):o1 * (BL // 2)],
                    osb_big[:, o0 * BL:o1 * BL].bitcast(f32))

            # ---------------- schedule ----------------
            front_full()
            u_tile(0)
            u_tile(1)
            l1_chunk(0, R_tiles[0][:, :], P)
            u_tile(2)
            l1_chunk(1, R_tiles[1][:, :], P)
            u_tile(3)
            l1_chunk(2, R_tiles[2][:, :], P)
            u_tile(4)
            l1_chunk(3, mix[0:94, :], 94)
            u_tile(5)
            l1_chunk(4, U_tiles[0][:, :], P)
            u_tile(6)
            l1_chunk(5, U_tiles[1][:, :], P)
            u_tile(7)
            for t in range(2, NT):
                l1_chunk(4 + t, U_tiles[t][:, :], P)
            mlp_head()
            mlp_l2()
            for o in range(NT):
                mlp_l3(o)
                if o % 2 == 1:
                    out_dma(o - 1, o + 1)

    nc.compile()
    _PROG_CACHE[key] = nc
    return nc


def _run(inputs, trace=False, tmpdir=None, **_kw):
    from concourse import bass_utils

    nc = _build_program()

    prep = _host_prep(inputs["q_idx"], inputs["fm_vars"],
                      inputs["w1"], inputs["b1"], inputs["w2"], inputs["b2"],
                      inputs["w3"], inputs["b3"])
    emb = np.ascontiguousarray(np.asarray(inputs["emb"], np.float32))
    stu_id = np.asarray(inputs["stu_id"]).astype(np.int32)

    in_maps = []
    for c in range(NCORES):
        sidx = np.ascontiguousarray(
            stu_id[c * BL:(c + 1) * BL].reshape(NG, P).T).astype(np.int32)
        in_maps.append(dict(emb=emb, sidx=sidx, **prep))

    if trace:
        import sys, types
        if "antenv.axon_hooks" not in sys.modules:
            import trn_agent_boot.trn_boot as tb
            mod = types.ModuleType("antenv.axon_hooks")
            hook = tb._ntff_profile_via_ctypes("/opt/axon/libaxon_pjrt.so")
            mod.get_axon_ntff_profile_hook = lambda: hook
            mod.set_axon_ntff_profile_hook = lambda h: None
            sys.modules["antenv.axon_hooks"] = mod
        bass_utils.upload_artifacts = lambda d: d

    res = bass_utils.run_bass_kernel_spmd(
        nc, in_maps, core_ids=list(range(NCORES)), trace=trace, tmpdir=tmpdir)

    outs = []
    for c in range(NCORES):
        arr = np.ascontiguousarray(res.results[c]["out"]).view(_np_f16())
        arr = arr.reshape(P, NT, BL)              # [p, o, b]
        arr = arr.transpose(2, 1, 0).reshape(BL, NOUT)      # [b, n]
        outs.append(arr)
    out = np.concatenate(outs, axis=0)
    return np.ascontiguousarray(out.astype(np.float32)), res


def kernel(**inputs):
    out, _ = _run(inputs, trace=False)
    return out

